# revision 16
# baseline (speedup 1.0000x reference)
"""Bass/Tile Trainium2 kernel for nn_CondRealNVPFlow3D (8-core SPMD).

The axon tunnel to the TRN2 cores runs at only ~5-30 MB/s with ~100 ms
dispatch latency, while the device compute for this problem is ~1 ms.  So the
kernel is organized to minimize wire bytes:

  * Host computes everything that does not touch the (B, N) point cloud:
    the FiLM MLPs (need only g and tiny weights; their BatchNorm uses the
    full 64-sample batch which the host has), and the BN1 statistics (5
    global moments of (p1, p2)) folded into the first SharedDot layer.
  * Device (data-parallel over batch, 8 samples/core) runs the fused
    dual-branch point MLP: h1 = relu(A @ [p1;p2] + c); h2 = W1bd @ h1;
    pass B accumulates per-channel stats of h2, one AllReduce of (128, 2)
    sums gives the global BatchNorm2 moments; pass C recomputes h2 and
    applies BN2+FiLM+ReLU (single fused ACT op), the final SharedDot and
    soft-sign.  All matmuls run in fp16 (PSUM accumulates fp32).
  * Wire traffic per call: up = ONE packed fp16 tensor per core (p slice +
    folded weights, 4.35 MB total -- a single sharded array, because every
    extra input array costs ~150 ms of per-transfer RPC overhead); down =
    fp16 (lv0, mu0) (4.19 MB).  The previous call's output device buffers
    are donated back as the next call's output operands, so no zero-buffers
    cross the wire.
  * Host assembles the full (64, 3, 16384) outputs: mu/logvar are zero
    outside channel 0, p_out = sqrt(EPS+exp(logvar))*p + mu.

kernel(**inputs) -> (p_out, mu, logvar), each (64, 3, 16384) float32.
"""

import contextlib

import numpy as np

B, C, N = 64, 3, 16384
F, G = 64, 512
NCORES = 8
BL = B // NCORES            # 8 samples per core
EPS = 1e-6
BN_EPS = 1e-5
NT = 512                    # points per tile
HALF = N // 2               # 8192
NTOT = B * N                # global BN count
NLOC = BL * N               # per-core points

_cached = {}


def _build_nc():
    import concourse.bacc as bacc
    import concourse.bass as bass
    import concourse.tile as tile
    from concourse import mybir

    f32 = mybir.dt.float32
    f16 = mybir.dt.float16
    AF = mybir.ActivationFunctionType
    ALU = mybir.AluOpType

    nc = bacc.Bacc("TRN2", target_bir_lowering=False, debug=False,
                   num_devices=NCORES)

    def din(name, shape, dt=f32):
        return nc.dram_tensor(name, list(shape), dt, kind="ExternalInput").ap()

    def dout(name, shape, dt=f32):
        return nc.dram_tensor(name, list(shape), dt, kind="ExternalOutput").ap()

    # Everything lives in ONE packed fp16 input tensor: each sharded input
    # array costs ~150+ ms of fixed per-transfer RPC overhead on the axon
    # tunnel, so 1 input beats 7.  Element offsets into pall:
    #   [0:KP)            p[:, 1:3, :] per-core slice, laid out (BL, 2, N)
    #   KP + [0:256)      lh0r (2,128)    KP + [256:384)    crow (128,)
    #   KP + [384:4480)   w1 lv block (64,64)   KP + [4480:8576) w1 mu block
    #   KP + [8576:8640)  lv_sd2_W (64,)  KP + [8640:8704)  mu_sd2_W (64,)
    #   KP + [8704:9728)  sfull.T (128,BL)   KP + [9728:10752) bfull.T (128,BL)
    #   KP + [10752:10754) sd2b (2,)
    KP = BL * 2 * N
    KTOT = KP + 10754
    pall_d = din("pall", (1, KTOT), f16)
    o2_d = dout("o2h", (BL, 2, N), f16)

    def wsl(a, b):
        return pall_d[0, KP + a:KP + b]

    with tile.TileContext(nc) as tc:
        st = contextlib.ExitStack()
        sing = st.enter_context(tc.tile_pool(name="sing", bufs=1))
        work = st.enter_context(tc.tile_pool(name="work", bufs=3))
        ps = st.enter_context(tc.tile_pool(name="ps", bufs=2, space="PSUM"))
        dram = st.enter_context(tc.tile_pool(name="dram", bufs=1, space="DRAM"))

        # ---------------- small weights (from the packed wpk) ----------------
        # lh0: BN1-folded first layer, quadrant-replicated at partition
        # bases 0/32/64/96 as rows [arow; brow] (128 out-ch = lv|mu).
        lh0 = sing.tile([128, 128], f16)
        for j in range(4):
            nc.sync.dma_start(out=lh0[32 * j:32 * j + 2, :],
                              in_=wsl(0, 256).rearrange("(p f) -> p f", f=128))
        crow16 = sing.tile([128, 1], f16)
        nc.sync.dma_start(out=crow16,
                          in_=wsl(256, 384).rearrange("(p f) -> p f", f=1))
        crow = sing.tile([128, 1], f32)
        nc.vector.tensor_copy(out=crow, in_=crow16)
        w1 = sing.tile([128, 128], f16)
        nc.vector.memset(w1, 0.0)
        nc.gpsimd.dma_start(out=w1[0:64, 0:64],
                            in_=wsl(384, 4480).rearrange("(p f) -> p f", f=64))
        nc.gpsimd.dma_start(out=w1[64:128, 64:128],
                            in_=wsl(4480, 8576).rearrange("(p f) -> p f", f=64))
        vbd = sing.tile([128, 2], f16)
        nc.vector.memset(vbd, 0.0)
        nc.gpsimd.dma_start(out=vbd[0:64, 0:1],
                            in_=wsl(8576, 8640).rearrange("(p f) -> p f", f=1))
        nc.gpsimd.dma_start(out=vbd[64:128, 1:2],
                            in_=wsl(8640, 8704).rearrange("(p f) -> p f", f=1))
        sfull16 = sing.tile([128, BL], f16)
        nc.sync.dma_start(out=sfull16,
                          in_=wsl(8704, 9728).rearrange("(p f) -> p f", f=BL))
        sfull = sing.tile([128, BL], f32)
        nc.vector.tensor_copy(out=sfull, in_=sfull16)
        bfull16 = sing.tile([128, BL], f16)
        nc.sync.dma_start(out=bfull16,
                          in_=wsl(9728, 10752).rearrange("(p f) -> p f", f=BL))
        bfull = sing.tile([128, BL], f32)
        nc.vector.tensor_copy(out=bfull, in_=bfull16)
        bneps = sing.tile([128, 1], f32)
        nc.vector.memset(bneps, float(BN_EPS))

        # ---------------- p loads ----------------
        # PU tiles: 4 static (128, HALF) f16 tiles; tile i holds units
        # u = 4*i + j at partition base 32*j as rows [p1; p2].
        # unit u = (sample, half) = divmod(u, 2).
        PU = []
        for i in range(4):
            t = sing.tile([128, HALF], f16, name=f"PU{i}")
            for j in range(4):
                u = 4 * i + j
                s, h = divmod(u, 2)
                nc.gpsimd.dma_start(out=t[32 * j:32 * j + 2, :],
                                    in_=bass.AP(tensor=pall_d.tensor,
                                                offset=s * 2 * N + h * HALF,
                                                ap=[[N, 2], [1, HALF]]))
            PU.append(t)

        # ---------------- PASS B: h2 channel stats ----------------
        stats = sing.tile([128, 256, 6], f32)
        tile_idx = 0
        for u in range(16):
            base = 32 * (u % 4)
            pt = PU[u // 4]
            for t in range(HALF // NT):
                ph1 = ps.tile([128, NT], f32, tag="ph1")
                nc.tensor.matmul(ph1, lh0[base:base + 2, :],
                                 pt[base:base + 2, t * NT:(t + 1) * NT],
                                 start=True, stop=True, tile_position=(base, 0))
                r = work.tile([128, NT], f16, tag="r")
                nc.scalar.activation(r, ph1, AF.Relu, bias=crow)
                ph2 = ps.tile([128, NT], f32, tag="ph2")
                nc.tensor.matmul(ph2, w1, r, start=True, stop=True)
                nc.vector.bn_stats(stats[:, tile_idx, :], ph2)
                tile_idx += 1
        assert tile_idx == 256

        mv2 = sing.tile([128, 2], f32)
        nc.vector.bn_aggr(mv2, stats)
        sq2 = sing.tile([128, 2], f32)
        nc.vector.tensor_scalar(out=sq2[:, 0:1], in0=mv2[:, 0:1],
                                scalar1=float(NLOC), scalar2=None, op0=ALU.mult)
        tq = sing.tile([128, 1], f32)
        nc.vector.tensor_tensor(out=tq, in0=mv2[:, 0:1], in1=mv2[:, 0:1], op=ALU.mult)
        nc.vector.tensor_tensor(out=tq, in0=tq, in1=mv2[:, 1:2], op=ALU.add)
        nc.vector.tensor_scalar(out=sq2[:, 1:2], in0=tq, scalar1=float(NLOC),
                                scalar2=None, op0=ALU.mult)
        cin2 = dram.tile([128, 2], f32)
        cout2 = dram.tile([128, 2], f32)
        nc.sync.dma_start(out=cin2, in_=sq2)
        nc.gpsimd.collective_compute(
            "AllReduce", ALU.add, replica_groups=[list(range(NCORES))],
            ins=[cin2[:, :]], outs=[cout2[:, :]])
        gq2 = sing.tile([128, 2], f32)
        nc.sync.dma_start(out=gq2, in_=cout2)
        inv_n = 1.0 / NTOT
        m2 = sing.tile([128, 1], f32)
        nc.vector.tensor_scalar(out=m2, in0=gq2[:, 0:1], scalar1=inv_n,
                                scalar2=None, op0=ALU.mult)
        v2 = sing.tile([128, 1], f32)
        nc.vector.tensor_tensor(out=v2, in0=m2, in1=m2, op=ALU.mult)
        q2m = sing.tile([128, 1], f32)
        nc.vector.tensor_scalar(out=q2m, in0=gq2[:, 1:2], scalar1=inv_n,
                                scalar2=None, op0=ALU.mult)
        nc.vector.tensor_tensor(out=v2, in0=q2m, in1=v2, op=ALU.subtract)
        rstd2 = sing.tile([128, 1], f32)
        nc.scalar.activation(rstd2, v2, AF.Sqrt, bias=bneps)
        nc.vector.reciprocal(out=rstd2, in_=rstd2)
        alpha = sing.tile([128, BL], f32)
        nc.vector.tensor_scalar(out=alpha, in0=sfull, scalar1=rstd2,
                                scalar2=None, op0=ALU.mult)
        beta = sing.tile([128, BL], f32)
        nc.vector.tensor_scalar(out=beta, in0=alpha, scalar1=m2, scalar2=None,
                                op0=ALU.mult)
        nc.vector.tensor_tensor(out=beta, in0=bfull, in1=beta, op=ALU.subtract)

        # ---------------- PASS C ----------------
        Lc = sing.tile([128, 1024], f32)
        Mc = sing.tile([128, 1024], f32)
        for u in range(16):
            s, h = divmod(u, 2)
            base = 32 * (u % 4)
            pt = PU[u // 4]
            for grp in range(4):
                cv = work.tile([2, 2048], f32, tag="cv", bufs=2)
                for pos in range(4):
                    t = grp * 4 + pos
                    ph1 = ps.tile([128, NT], f32, tag="ph1")
                    nc.tensor.matmul(ph1, lh0[base:base + 2, :],
                                     pt[base:base + 2, t * NT:(t + 1) * NT],
                                     start=True, stop=True, tile_position=(base, 0))
                    r = work.tile([128, NT], f16, tag="r")
                    nc.scalar.activation(r, ph1, AF.Relu, bias=crow)
                    ph2 = ps.tile([128, NT], f32, tag="ph2")
                    nc.tensor.matmul(ph2, w1, r, start=True, stop=True)
                    q = work.tile([128, NT], f16, tag="q")
                    nc.scalar.activation(q, ph2, AF.Relu,
                                         bias=beta[:, s:s + 1], scale=alpha[:, s:s + 1])
                    ov = ps.tile([2, NT], f32, tag="ov")
                    nc.tensor.matmul(ov, vbd, q, start=True, stop=True)
                    dst = cv[:, pos * NT:(pos + 1) * NT]
                    if pos % 2 == 0:
                        nc.vector.tensor_copy(out=dst, in_=ov)
                    else:
                        nc.scalar.copy(out=dst, in_=ov)
                # repack: tiles t0..t0+3 (t0 = 16*h + 4*grp) -> rows of Lc/Mc
                t0 = 16 * h + 4 * grp
                prt = s * 16 + t0 // 2
                dl = Lc[prt:prt + 2, :].rearrange("p (g f) -> p g f", f=NT)
                dm = Mc[prt:prt + 2, :].rearrange("p (g f) -> p g f", f=NT)
                nc.sync.dma_start(out=dl, in_=cv[0:1, :].rearrange("p (g f) -> p g f", f=NT))
                nc.sync.dma_start(out=dm, in_=cv[1:2, :].rearrange("p (g f) -> p g f", f=NT))

        # ---------------- final math ----------------
        sd2bL16 = sing.tile([128, 1], f16)
        nc.gpsimd.dma_start(out=sd2bL16, in_=bass.AP(
            tensor=pall_d.tensor, offset=KP + 10752, ap=[[0, 128], [1, 1]]))
        sd2bM16 = sing.tile([128, 1], f16)
        nc.gpsimd.dma_start(out=sd2bM16, in_=bass.AP(
            tensor=pall_d.tensor, offset=KP + 10753, ap=[[0, 128], [1, 1]]))
        sd2bL = sing.tile([128, 1], f32)
        nc.vector.tensor_copy(out=sd2bL, in_=sd2bL16)
        sd2bM = sing.tile([128, 1], f32)
        nc.vector.tensor_copy(out=sd2bM, in_=sd2bM16)
        nc.vector.tensor_scalar(out=Lc, in0=Lc, scalar1=sd2bL, scalar2=None, op0=ALU.add)
        nc.vector.tensor_scalar(out=Mc, in0=Mc, scalar1=sd2bM, scalar2=None, op0=ALU.add)
        ab = sing.tile([128, 1024], f32)
        nc.scalar.activation(ab, Lc, AF.Abs)
        nc.vector.tensor_scalar(out=ab, in0=ab, scalar1=1.0, scalar2=None, op0=ALU.add)
        nc.vector.reciprocal(out=ab, in_=ab)
        Lh = sing.tile([128, 1024], f16)
        nc.vector.tensor_tensor(out=Lh, in0=Lc, in1=ab, op=ALU.mult)
        Mh = sing.tile([128, 1024], f16)
        nc.scalar.copy(out=Mh, in_=Mc)

        # ---------------- output DMAs ----------------
        for s_ in range(BL):
            r16 = slice(s_ * 16, (s_ + 1) * 16)
            nc.sync.dma_start(out=o2_d[s_, 0, :].rearrange("(t f) -> t f", f=1024),
                              in_=Lh[r16, :])
            nc.sync.dma_start(out=o2_d[s_, 1, :].rearrange("(t f) -> t f", f=1024),
                              in_=Mh[r16, :])
        st.close()

    nc.compile()
    return nc


def _film_host(g, W0, bn_g, bn_b, W1, b1):
    h = g @ W0.T
    m = h.mean(axis=0)
    v = h.var(axis=0)
    hn = (h - m) / np.sqrt(v + BN_EPS) * bn_g + bn_b
    hs = hn / (1.0 + np.exp(-hn))          # Swish: hn * sigmoid(hn)
    return hs @ W1.T + b1


def _prep_host(inputs):
    f = np.float32
    p = np.asarray(inputs["p"], f)
    g = np.asarray(inputs["g"], f)

    def W(name):
        return np.asarray(inputs[name], f)

    # FiLM conditioning (replicated on host; BN over the full 64 batch)
    wf, bf = [], []
    for pre in ("lv", "mu"):
        wf.append(_film_host(g, W(pre + "_cw_W0"), W(pre + "_cw_bn_g"),
                             W(pre + "_cw_bn_b"), W(pre + "_cw_W1"), W(pre + "_cw_b1")))
        bf.append(_film_host(g, W(pre + "_cb_W0"), W(pre + "_cb_bn_g"),
                             W(pre + "_cb_bn_b"), W(pre + "_cb_W1"), W(pre + "_cb_b1")))
    sfull = EPS + np.exp(np.concatenate(wf, axis=1))   # (64, 128)
    bfull = np.concatenate(bf, axis=1)                 # (64, 128)

    # BN1 folded into the first SharedDot: 5 global moments of (p1, p2)
    p1 = p[:, 1, :]
    p2 = p[:, 2, :]
    e1 = p1.mean(dtype=np.float64)
    e2 = p2.mean(dtype=np.float64)
    q1 = np.mean(p1 * p1, dtype=np.float64)
    q2 = np.mean(p2 * p2, dtype=np.float64)
    e12 = np.mean(p1 * p2, dtype=np.float64)
    V1 = q1 - e1 * e1
    V2 = q2 - e2 * e2
    C12 = e12 - e1 * e2
    a = np.concatenate([W("lv_sd0_W")[:, 0], W("mu_sd0_W")[:, 0]]).astype(np.float64)
    b = np.concatenate([W("lv_sd0_W")[:, 1], W("mu_sd0_W")[:, 1]]).astype(np.float64)
    g1 = np.concatenate([W("lv_sd0_bn_g"), W("mu_sd0_bn_g")]).astype(np.float64)
    b1 = np.concatenate([W("lv_sd0_bn_b"), W("mu_sd0_bn_b")]).astype(np.float64)
    m1 = a * e1 + b * e2
    v1 = a * a * V1 + 2.0 * a * b * C12 + b * b * V2
    grs = g1 / np.sqrt(v1 + BN_EPS)
    arow = (a * grs).astype(f)
    brow = (b * grs).astype(f)
    crow = (b1 - grs * m1).astype(f)

    sd2b = np.array([[W("lv_sd2_b")[0], W("mu_sd2_b")[0]]], f)

    # single packed per-core input (offsets documented in _build_nc)
    KP = BL * 2 * N
    pall = np.empty((NCORES, KP + 10754), np.float16)
    pall[:, :KP].reshape(NCORES, BL, 2, N)[...] = \
        p.reshape(NCORES, BL, 3, N)[:, :, 1:3, :]
    wpk = pall[:, KP:]
    wpk[:, 0:256] = np.stack([arow, brow]).ravel()
    wpk[:, 256:384] = crow
    wpk[:, 384:4480] = W("lv_sd1_W").T.ravel()
    wpk[:, 4480:8576] = W("mu_sd1_W").T.ravel()
    wpk[:, 8576:8640] = W("lv_sd2_W")[0]
    wpk[:, 8640:8704] = W("mu_sd2_W")[0]
    for c in range(NCORES):
        sl = slice(c * BL, (c + 1) * BL)
        wpk[c, 8704:9728] = sfull[sl].T.astype(np.float16).ravel()
        wpk[c, 9728:10752] = bfull[sl].T.astype(np.float16).ravel()
    wpk[:, 10752:10754] = sd2b[0]

    # global (concat over 8 cores on axis 0) device input arrays;
    # per-core shape (1, KTOT)
    arrs = {"pall": pall}
    return p, arrs


def _get_runner():
    """Build once; cache a jitted shard_map callable (avoids per-call retrace)."""
    if "run" in _cached:
        return _cached["run"]
    import jax
    import numpy as _np
    from jax.sharding import Mesh, PartitionSpec
    from jax.experimental.shard_map import shard_map
    from concourse import mybir
    from concourse import bass2jax
    from concourse.bass2jax import _bass_exec_p, install_neuronx_cc_hook

    nc = _build_nc()
    _cached["nc"] = nc
    install_neuronx_cc_hook()

    partition_name = nc.partition_id_tensor.name if nc.partition_id_tensor else None
    in_names, out_names, out_avals, zero_outs = [], [], [], []
    for alloc in nc.m.functions[0].allocations:
        if not isinstance(alloc, mybir.MemoryLocationSet):
            continue
        name = alloc.memorylocations[0].name
        if alloc.kind == "ExternalInput":
            if name != partition_name:
                in_names.append(name)
        elif alloc.kind == "ExternalOutput":
            out_names.append(name)
            shape = tuple(alloc.tensor_shape)
            dtype = mybir.dt.np(alloc.dtype)
            out_avals.append(jax.core.ShapedArray(shape, dtype))
            zero_outs.append(_np.zeros(shape, dtype))
    n_params = len(in_names)
    all_names = in_names + out_names
    if partition_name is not None:
        all_names = all_names + [partition_name]

    def _body(*args):
        operands = list(args)
        if partition_name is not None:
            operands.append(bass2jax.partition_id_tensor())
        outs = _bass_exec_p.bind(
            *operands,
            out_avals=tuple(out_avals),
            in_names=tuple(all_names),
            out_names=tuple(out_names),
            lowering_input_output_aliases=(),
            sim_require_finite=True,
            sim_require_nnan=True,
            nc=nc,
        )
        return tuple(outs)

    devices = jax.devices()[:NCORES]
    mesh = Mesh(_np.asarray(devices), ("core",))
    n_outs = len(out_names)
    sharded = jax.jit(
        shard_map(_body, mesh=mesh,
                  in_specs=(PartitionSpec("core"),) * (n_params + n_outs),
                  out_specs=(PartitionSpec("core"),) * n_outs),
        donate_argnums=tuple(range(n_params, n_params + n_outs)),
        keep_unused=True,
    )

    from concurrent.futures import ThreadPoolExecutor
    pool = ThreadPoolExecutor(NCORES)
    _cached["pool"] = pool

    def pull(arr):
        # per-shard parallel D2H: noticeably faster than np.asarray when the
        # tunnel is in a slow phase, neutral otherwise
        try:
            out = _np.empty(arr.shape, arr.dtype)
            shards = arr.addressable_shards

            def fetch(s):
                out[s.index] = _np.asarray(s.data)

            list(pool.map(fetch, shards))
            return out
        except Exception:
            return _np.asarray(arr)

    def run(arrs):
        import os
        import time as _time
        ins = [arrs[nm] for nm in in_names]
        prev = _cached.get("prev_outs")
        if prev is None:
            prev = [_np.zeros((NCORES * z.shape[0], *z.shape[1:]), z.dtype)
                    for z in zero_outs]
        prof = os.environ.get("KPROF")
        t0 = _time.perf_counter()
        out_arrs = sharded(*ins, *prev)
        if prof:
            t1 = _time.perf_counter()
            for o in out_arrs:
                o.block_until_ready()
            t2 = _time.perf_counter()
        res = {nm: pull(out_arrs[i]) for i, nm in enumerate(out_names)}
        if prof:
            t3 = _time.perf_counter()
            print(f"KPROF dispatch {t1 - t0:.3f}s  exec/stage {t2 - t1:.3f}s  "
                  f"pull {t3 - t2:.3f}s", flush=True)
        _cached["prev_outs"] = list(out_arrs)
        return res

    _cached["run"] = run
    return run


def kernel(**inputs):
    run = _get_runner()
    p, arrs = _prep_host(inputs)
    res = run(arrs)
    o = res["o2h"]                                 # (64, 2, 16384) fp16
    f = np.float32
    lv0 = o[:, 0, :].astype(f)
    mu0 = o[:, 1, :].astype(f)
    logvar = np.zeros((B, C, N), f)
    logvar[:, 0, :] = lv0
    mu = np.zeros((B, C, N), f)
    mu[:, 0, :] = mu0
    p_out = np.empty((B, C, N), f)
    p_out[:, 0, :] = np.sqrt(EPS + np.exp(lv0)) * p[:, 0, :] + mu0
    p_out[:, 1:3, :] = p[:, 1:3, :] * f(np.sqrt(1.0 + EPS))
    return p_out, mu, logvar
